# revision 44
# baseline (speedup 1.0000x reference)
"""Causal multi-head self-attention on 8 Trainium2 NeuronCores.

Sharding: 4 batches x 2 head-groups (8 heads each). Core c = (b, g) with
b = c // 2, g = c % 2. Each core computes QKV projections for its weight
row-slice, attention for its 8 heads, and a partial out-projection
(Megatron row-parallel). Host sums the two partials per batch and adds bo.

All shapes hardcoded for x [4, 2048, 1024], 16 heads, head_dim 64, fp32.

Schedule: engine queues run in emission order, so the QKV projection of
sblock sb+1 and the out-projection of sblock sb-1 are emitted interleaved
into the (scalar-bound) attention groups of sblock sb to fill tensor-engine
gaps. Softmax 1/Z uses reciprocal_approx_fast + DMA partition-broadcast.
Exp is fused over 2-key-chunk score groups. The two heads of a pair run
their K=64 score matmuls concurrently on disjoint PE row halves.
"""

import sys
import numpy as np

if "/opt/trn_rl_repo" not in sys.path:
    sys.path.insert(0, "/opt/trn_rl_repo")

B = 4
S = 2048
D = 1024
HG = 2            # head groups (cores per batch)
NHL = 8           # heads per core
DH = 64
DG = NHL * DH     # 512 feature dims per core
SB = 512          # s-block
NSB = S // SB     # 4
NEG = -1.0e9
SCALE = 0.125     # 1/sqrt(64)

# normalization path: "fast" = reciprocal_approx_fast + gpsimd partition
# broadcast; "mm" = exact reciprocal + matmul broadcast (baseline style)
FAST_RECIP = True
FAST_BCAST = True

_CACHE = {}


def _build_nc():
    import concourse.bass as bass
    import concourse.bacc as bacc
    import concourse.tile as tile
    from concourse import mybir
    from contextlib import ExitStack

    f32 = mybir.dt.float32
    bf16 = mybir.dt.bfloat16
    AF = mybir.ActivationFunctionType
    ts = bass.ts

    nc = bacc.Bacc(None, target_bir_lowering=False)

    # tile-major DRAM layouts: each SBUF tile loads as ONE contiguous DMA
    # with 8 KB per-partition lines (fragmented layouts cost ~20 us startup)
    xt_d = nc.dram_tensor("xt", [NSB, 128, 8, SB], bf16, kind="ExternalInput")
    wqt_d = nc.dram_tensor("wqt", [128, 8, DG], bf16, kind="ExternalInput")
    wkt_d = nc.dram_tensor("wkt", [128, 8, DG], bf16, kind="ExternalInput")
    wvt_d = nc.dram_tensor("wvt", [128, 8, DG], bf16, kind="ExternalInput")
    wot_d = nc.dram_tensor("wot", [128, 4, D], bf16, kind="ExternalInput")
    bqc_d = nc.dram_tensor("bqc", [128, 4], f32, kind="ExternalInput")
    bkc_d = nc.dram_tensor("bkc", [128, 4], f32, kind="ExternalInput")
    bvb_d = nc.dram_tensor("bvb", [128, NHL, DH], f32, kind="ExternalInput")
    # outputs in bf16: halves the output DMA bytes; host sums/upcasts
    out_d = nc.dram_tensor("out", [S, D], bf16, kind="ExternalOutput")
    # per-pair out-projection partials for the last sblock (pairs 0-2);
    # the host adds them into out rows [1536:2048]
    o3p_d = nc.dram_tensor("o3p", [3, SB, D], bf16, kind="ExternalOutput")

    with tile.TileContext(nc) as tc, ExitStack() as ctx:
        consts = ctx.enter_context(tc.tile_pool(name="consts", bufs=1))
        cache = ctx.enter_context(tc.tile_pool(name="cache", bufs=1))
        xt_pool = ctx.enter_context(tc.tile_pool(name="xtp", bufs=2))
        qt_pool = ctx.enter_context(tc.tile_pool(name="qtp", bufs=2))
        work = ctx.enter_context(tc.tile_pool(name="work", bufs=1))
        ppool = ctx.enter_context(tc.tile_pool(name="pp", bufs=2, space="PSUM"))
        pscore = ctx.enter_context(tc.tile_pool(name="ps", bufs=2, space="PSUM"))
        pout2 = ctx.enter_context(tc.tile_pool(name="po", bufs=2, space="PSUM"))

        # ---- x for sblock 0 first so the first projection can start early ----
        xt_tiles = {}

        # spread input DMAs over several engines' DMA queues so the startup
        # load (~5 MB) doesn't serialize on one queue
        dmaq = [nc.sync, nc.scalar, nc.gpsimd]

        def load_xt(sb):
            xt_tiles[sb] = xt_pool.tile(
                [128, 8, SB], bf16, tag="xt", name=f"xt{sb}"
            )
            nc.sync.dma_start(xt_tiles[sb], xt_d[sb, :, :, :])

        # startup is DMA-bandwidth-bound: issue the critical tensors first
        # (xt0 + wk feed the first K-projection chains), split into ec-pieces
        # so the first chain starts after the first 2-chunk piece lands.
        xt_tiles[0] = xt_pool.tile([128, 8, SB], bf16, tag="xt", name="xt0")
        wk_t = consts.tile([128, 8, DG], bf16)
        wq_t = consts.tile([128, 8, DG], bf16)
        wv_t = consts.tile([128, 8, DG], bf16)
        for e0 in range(0, 8, 2):
            nc.sync.dma_start(xt_tiles[0][:, e0 : e0 + 2, :], xt_d[0, :, e0 : e0 + 2, :])
            nc.scalar.dma_start(wk_t[:, e0 : e0 + 2, :], wkt_d[:, e0 : e0 + 2, :])
        bkc_t = consts.tile([128, 4], f32)
        nc.gpsimd.dma_start(bkc_t, bkc_d[:, :])
        bqc_t = consts.tile([128, 4], f32)
        nc.gpsimd.dma_start(bqc_t, bqc_d[:, :])
        for e0 in range(0, 8, 2):
            nc.scalar.dma_start(wq_t[:, e0 : e0 + 2, :], wqt_d[:, e0 : e0 + 2, :])
        for e0 in range(0, 8, 2):
            nc.sync.dma_start(wv_t[:, e0 : e0 + 2, :], wvt_d[:, e0 : e0 + 2, :])
        bv_bc = consts.tile([128, NHL, DH], f32)
        nc.gpsimd.dma_start(bv_bc, bvb_d[:, :, :])
        # wo is first needed by the out-projection of sblock 0, deferred into
        # sblock 2's attention — load it once the startup burst has drained
        wo_t = consts.tile([128, 4, D], bf16)

        def load_wo():
            nc.gpsimd.dma_start(wo_t, wot_d[:, :, :])

        ones64 = consts.tile([65, 64], bf16)
        nc.any.memset(ones64, 1.0)

        # diag mask: m128[tt, c] = 0 if c >= tt else NEG
        m128 = consts.tile([128, 128], f32)
        nc.any.memset(m128, 0.0)
        nc.gpsimd.affine_select(
            out=m128,
            in_=m128,
            compare_op=mybir.AluOpType.is_ge,
            fill=NEG,
            base=0,
            pattern=[[1, 128]],
            channel_multiplier=-1,
        )
        # bf16 copies for adding the mask via a tensor-engine accumulation
        # (identity.T @ m128) instead of a vector add on the exp critical path
        m128b = consts.tile([128, 128], bf16)
        nc.vector.tensor_copy(m128b, m128)
        ident = consts.tile([128, 128], f32)
        nc.any.memset(ident, 1.0)
        nc.gpsimd.affine_select(
            out=ident, in_=ident, compare_op=mybir.AluOpType.is_ge,
            fill=0.0, base=0, pattern=[[1, 128]], channel_multiplier=-1,
        )
        nc.gpsimd.affine_select(
            out=ident, in_=ident, compare_op=mybir.AluOpType.is_ge,
            fill=0.0, base=0, pattern=[[-1, 128]], channel_multiplier=1,
        )
        identb = consts.tile([128, 128], bf16)
        nc.vector.tensor_copy(identb, ident)
        # dummy exp to pull the ACT table load into the startup phase
        exwarm = consts.tile([1, 8], bf16)
        nc.scalar.activation(exwarm, m128[0:1, 0:8], AF.Exp, scale=SCALE)

        # ---- persistent K/V caches ----
        kt_all = cache.tile([128, 4, S], bf16)       # [d within pair chunk, pair, t]
        v_aug = cache.tile([128, 16, NHL, DH + 1], bf16)  # [t in chunk, tchunk, head, d|1]
        nc.any.memset(v_aug[:, :, :, DH : DH + 1], 1.0)

        qt_tiles = {}

        def proj_closures(sb):
            """12 tensor-work closures: Q, K (4 dim-chunks each), V (4 t-chunks)."""
            s0 = sb * SB
            qt_tiles[sb] = qt_pool.tile([128, 4, SB], bf16, tag="qt", name=f"qt{sb}")
            xt_sb = xt_tiles[sb]
            items = []

            def q_group(dc):
                def emit():
                    pq = ppool.tile([128, SB], f32, tag="pp")
                    for ec in range(8):
                        nc.tensor.matmul(
                            pq, wq_t[:, ec, ts(dc, 128)], xt_sb[:, ec, :],
                            start=(ec == 0), stop=(ec == 7),
                        )
                    nc.vector.tensor_scalar_add(
                        qt_tiles[sb][:, dc, :], pq, bqc_t[:, dc : dc + 1]
                    )
                return emit

            def k_group(dc):
                def emit():
                    pk = ppool.tile([128, SB], f32, tag="pp")
                    for ec in range(8):
                        nc.tensor.matmul(
                            pk, wk_t[:, ec, ts(dc, 128)], xt_sb[:, ec, :],
                            start=(ec == 0), stop=(ec == 7),
                        )
                    nc.vector.tensor_scalar_add(
                        kt_all[:, dc, s0 : s0 + SB], pk, bkc_t[:, dc : dc + 1]
                    )
                return emit

            def v_group(tsub):
                def emit():
                    tcg = 4 * sb + tsub
                    pv = ppool.tile([128, NHL, DH], f32, tag="pp")
                    for ec in range(8):
                        nc.tensor.matmul(
                            pv, xt_sb[:, ec, ts(tsub, 128)], wv_t[:, ec, :],
                            start=(ec == 0), stop=(ec == 7),
                        )
                    nc.vector.tensor_add(v_aug[:, tcg, :, 0:DH], pv, bv_bc)
                return emit

            # K first (needed by scores of the first groups), then Q, then V
            for dc in range(4):
                items.append(k_group(dc))
            for dc in range(4):
                items.append(q_group(dc))
            for tsub in range(4):
                items.append(v_group(tsub))
            return items

        ao_tiles_by_sb = {}

        def outproj_closures(sb):
            """8 tensor-work closures: 4 s-chunks x 2 output halves."""
            s0 = sb * SB
            ao_tiles = ao_tiles_by_sb[sb]
            items = []

            def o_group(sc, oh):
                def emit():
                    po = ppool.tile([128, 512], f32, tag="pp")
                    for p in range(4):
                        nc.tensor.matmul(
                            po,
                            ao_tiles[p][:, ts(sc, 128)],
                            wo_t[:, p, ts(oh, 512)],
                            start=(p == 0), stop=(p == 3),
                        )
                    po_sb = work.tile([128, 512], bf16, tag="posb", bufs=4)
                    nc.vector.tensor_copy(po_sb, po)
                    nc.sync.dma_start(
                        out_d[s0 + 128 * sc : s0 + 128 * (sc + 1), ts(oh, 512)], po_sb
                    )
                return emit

            for sc in range(4):
                for oh in range(2):
                    items.append(o_group(sc, oh))
            return items

        # sblock 0 projections run up front
        for it in proj_closures(0):
            it()

        for sb in range(NSB):
            s0 = sb * SB
            nkc = 4 * sb + 4
            ngrp = nkc // 2
            qt_sb = qt_tiles[sb]

            # deferred tensor work to interleave into this sblock's attention
            # rebalance: early sblocks are tensor-rich (projections), late
            # sblocks scalar-rich (attention) — push out-projections two
            # sblocks later so sb3's 32 groups get tensor filler.
            deferred = []
            if sb == 0:
                load_wo()
            if sb + 1 < NSB:
                load_xt(sb + 1)
                deferred += proj_closures(sb + 1)
            if sb == 2:
                deferred += outproj_closures(0)
            elif sb == 3:
                deferred += outproj_closures(1)
                deferred += outproj_closures(2)
            total_groups = ngrp * 4
            emitted = [0]
            gidx = [0]
            # reserve up to 2 items per pair boundary: the normalization
            # chain there leaves the PE idle long enough to re-throttle HAM
            n_boundary = min(len(deferred), 8)
            n_paced = len(deferred) - n_boundary

            def pace():
                gidx[0] += 1
                want = n_paced * gidx[0] // total_groups
                while emitted[0] < want:
                    deferred[emitted[0]]()
                    emitted[0] += 1

            def pace_boundary():
                for _ in range(2):
                    if emitted[0] < len(deferred):
                        deferred[emitted[0]]()
                        emitted[0] += 1

            def flush_deferred():
                while emitted[0] < len(deferred):
                    deferred[emitted[0]]()
                    emitted[0] += 1

            # ---- attention, per head-pair; key chunks in groups of 2 ----
            for p in range(4):
                out2 = [
                    pout2.tile([DH + 1, SB], f32, tag="po", name=f"out2_{hh}")
                    for hh in range(2)
                ]
                prev = None  # (ex tiles per hh, [c0 per chunk], [kc per chunk])
                for g in range(ngrp):
                    kcs = [2 * g, 2 * g + 1]
                    c0s = [max(0, 128 * (kc - 4 * sb)) for kc in kcs]
                    diag = kcs[1] >= 4 * sb
                    cur_st = []
                    cur_ex = []
                    # scores: 2 chunks x 2 heads; the two heads' matmuls use
                    # disjoint PE row halves (K=64) and run concurrently.
                    for hh in range(2):
                        r0 = 64 * hh
                        st = pscore.tile([128, 2, SB], f32, tag="ps")
                        cur_st.append(st)
                        for i in range(2):
                            nc.tensor.matmul(
                                st[:, i, c0s[i] : SB],
                                kt_all[r0 : r0 + 64, p, ts(kcs[i], 128)],
                                qt_sb[r0 : r0 + 64, p, c0s[i] : SB],
                                start=True, stop=not diag,
                                tile_position=(r0, 0),
                            )
                            if diag:
                                # causal mask via accumulation: += I.T @ m128
                                nc.tensor.matmul(
                                    st[:, i, c0s[i] : c0s[i] + 128],
                                    identb, m128b,
                                    start=False, stop=True,
                                )
                    # interleave: attn@V for the previous group
                    if prev is not None:
                        pex, pc0s, pkcs = prev
                        for i in range(2):
                            for hh in range(2):
                                nc.tensor.matmul(
                                    out2[hh][:, pc0s[i] : SB],
                                    v_aug[:, pkcs[i], 2 * p + hh, :],
                                    pex[hh][:, i, pc0s[i] : SB],
                                    start=(pkcs[i] == 0), stop=False,
                                )
                    for hh in range(2):
                        st = cur_st[hh]
                        ex = work.tile([128, 2, SB], bf16, tag="expt", bufs=4)
                        if diag:
                            for i in range(2):
                                nc.scalar.activation(
                                    ex[:, i, c0s[i] : SB],
                                    st[:, i, c0s[i] : SB],
                                    AF.Exp, scale=SCALE,
                                )
                        else:
                            nc.scalar.activation(
                                ex[:, :, :], st[:, :, :], AF.Exp, scale=SCALE
                            )
                        cur_ex.append(ex)
                    prev = (cur_ex, c0s, kcs)
                    pace()
                # final attn@V for the last group
                pex, pc0s, pkcs = prev
                for i in range(2):
                    for hh in range(2):
                        nc.tensor.matmul(
                            out2[hh][:, pc0s[i] : SB],
                            v_aug[:, pkcs[i], 2 * p + hh, :],
                            pex[hh][:, i, pc0s[i] : SB],
                            start=(pkcs[i] == 0), stop=(i == 1),
                        )
                if p == 3:
                    # everything left must land before the serial tail
                    flush_deferred()

                # ---- normalization: 1/Z (fast approx), DMA partition-bcast, mul
                # phases interleaved across the two heads so vector and
                # gpsimd pipeline instead of serializing the chain twice
                ao_p = work.tile([128, SB], bf16, tag=f"ao{p}", bufs=3)
                zcps, rzs, bcs = [], [], []
                for hh in range(2):
                    # stage Z into SBUF at partition 0: the custom-DVE ucode
                    # mishandles PSUM / nonzero-base inputs
                    zcp = work.tile([1, SB], f32, tag=f"zcp{hh}", bufs=2)
                    nc.vector.tensor_copy(zcp[0:1, :], out2[hh][DH : DH + 1, :])
                    zcps.append(zcp)
                for hh in range(2):
                    rz = work.tile([1, SB], f32, tag=f"rz{hh}", bufs=2)
                    nc.vector.reciprocal_approx_fast(
                        out=rz[0:1, :], in_=zcps[hh][0:1, :]
                    )
                    rzs.append(rz)
                for hh in range(2):
                    bc = work.tile([64, SB], f32, tag=f"bc{hh}", bufs=2)
                    nc.gpsimd.partition_broadcast(bc, rzs[hh][0:1, :])
                    bcs.append(bc)
                nc.vector.tensor_mul(ao_p[0:64, :], out2[0][0:DH, :], bcs[0])
                aotmp = work.tile([64, SB], bf16, tag="aotmp", bufs=2)
                nc.vector.tensor_mul(aotmp, out2[1][0:DH, :], bcs[1])
                nc.gpsimd.dma_start(ao_p[64:128, :], aotmp)
                if p == 0:
                    ao_tiles_by_sb[sb] = []
                ao_tiles_by_sb[sb].append(ao_p)
                if p < 3:
                    pace_boundary()
                if sb == NSB - 1:
                    # last sblock: out-projection per pair, right after the
                    # pair's normalization — kills the serial tail. Pairs
                    # 0-2 go to o3p_d; the host sums them into out.
                    for sc in range(4):
                        for oh in range(2):
                            po = ppool.tile([128, 512], f32, tag="pp")
                            nc.tensor.matmul(
                                po,
                                ao_p[:, ts(sc, 128)],
                                wo_t[:, p, ts(oh, 512)],
                                start=True, stop=True,
                            )
                            po_sb = work.tile([128, 512], bf16, tag="posb", bufs=4)
                            if oh == 1:
                                # split evacuation across scalar and vector so
                                # the psum-buffer rotation doesn't drip at
                                # single-engine copy pace
                                nc.scalar.copy(po_sb, po)
                            else:
                                nc.vector.tensor_copy(po_sb, po)
                            if p == 3:
                                nc.sync.dma_start(
                                    out_d[s0 + 128 * sc : s0 + 128 * (sc + 1),
                                          ts(oh, 512)],
                                    po_sb,
                                )
                            else:
                                nc.gpsimd.dma_start(
                                    o3p_d[p, 128 * sc : 128 * (sc + 1), ts(oh, 512)],
                                    po_sb,
                                )

            # any stragglers not yet emitted by pacing
            while emitted[0] < len(deferred):
                deferred[emitted[0]]()
                emitted[0] += 1



    nc.compile()
    return nc


def _prepare_core_inputs(x, Wq, bq, Wk, bk, Wv, bv):
    """Build per-core input maps. Core c: b = c // 2, g = c % 2."""
    import ml_dtypes

    BF = ml_dtypes.bfloat16

    def tilemajor(wT):
        # [D, F] -> [128, D//128, F]: partition-major so each SBUF tile
        # loads as one contiguous DMA
        F = wT.shape[1]
        return np.ascontiguousarray(
            wT.reshape(-1, 128, F).transpose(1, 0, 2)
        ).astype(BF)

    maps = []
    # x^T [1024, 2048] -> [NSB, 128, 8, SB]
    xt = []
    for b in range(B):
        xT = x[b].T  # [D, S]
        xt.append(np.ascontiguousarray(
            xT.reshape(8, 128, NSB, SB).transpose(2, 1, 0, 3)
        ).astype(BF))
    wq_s, wk_s, wv_s, bq_s, bk_s, bv_s = [], [], [], [], [], []
    for g in range(HG):
        sl = slice(g * DG, (g + 1) * DG)
        wq_s.append(tilemajor(Wq[sl, :].T))
        wk_s.append(tilemajor(Wk[sl, :].T))
        wv_s.append(tilemajor(Wv[sl, :].T))
        # Q/K biases in per-partition column layout [128, 4]: bias for dim
        # 128*dc + d sits at [d, dc].
        bq_s.append(np.ascontiguousarray(bq[sl].reshape(4, 128).T).astype(np.float32))
        bk_s.append(np.ascontiguousarray(bk[sl].reshape(4, 128).T).astype(np.float32))
        # V bias pre-broadcast across partitions: [128, NHL, DH]
        bv_s.append(
            np.ascontiguousarray(
                np.broadcast_to(bv[sl].reshape(1, NHL, DH), (128, NHL, DH))
            ).astype(np.float32)
        )
    for c in range(B * HG):
        b, g = c // HG, c % HG
        maps.append({
            "xt": xt[b],
            "wqt": wq_s[g], "wkt": wk_s[g], "wvt": wv_s[g],
            "wot": None,  # filled by caller (needs Wo)
            "bqc": bq_s[g], "bkc": bk_s[g], "bvb": bv_s[g],
        })
    return maps


def kernel(x, Wq, bq, Wk, bk, Wv, bv, Wo, bo):
    from concourse.bass_utils import run_bass_kernel_spmd

    x = np.asarray(x, dtype=np.float32)
    Wq, bq = np.asarray(Wq, np.float32), np.asarray(bq, np.float32)
    Wk, bk = np.asarray(Wk, np.float32), np.asarray(bk, np.float32)
    Wv, bv = np.asarray(Wv, np.float32), np.asarray(bv, np.float32)
    Wo, bo = np.asarray(Wo, np.float32), np.asarray(bo, np.float32)

    if "nc" not in _CACHE:
        _CACHE["nc"] = _build_nc()
    nc = _CACHE["nc"]

    import ml_dtypes

    maps = _prepare_core_inputs(x, Wq, bq, Wk, bk, Wv, bv)
    wot = [
        np.ascontiguousarray(
            Wo.T[g * DG : (g + 1) * DG, :]
            .reshape(4, 128, D)
            .transpose(1, 0, 2)
        ).astype(ml_dtypes.bfloat16)
        for g in range(HG)
    ]
    for c in range(B * HG):
        maps[c]["wot"] = wot[c % HG]

    res = run_bass_kernel_spmd(nc, maps, list(range(B * HG)))
    _CACHE["last_results"] = res

    out = np.empty((B, S, D), dtype=np.float32)
    for b in range(B):
        r0, r1 = res.results[2 * b], res.results[2 * b + 1]
        out[b] = (
            r0["out"].astype(np.float32) + r1["out"].astype(np.float32) + bo
        )
        out[b][3 * SB :] += (
            r0["o3p"].astype(np.float32).sum(axis=0)
            + r1["o3p"].astype(np.float32).sum(axis=0)
        )
    return out


# revision 46
# speedup vs baseline: 1.0592x; 1.0592x over previous
"""Causal multi-head self-attention on 8 Trainium2 NeuronCores.

Sharding: 4 batches x 2 head-groups (8 heads each). Core c = (b, g) with
b = c // 2, g = c % 2. Each core computes QKV projections for its weight
row-slice, attention for its 8 heads, and a partial out-projection
(Megatron row-parallel). Host sums the two partials per batch and adds bo.

All shapes hardcoded for x [4, 2048, 1024], 16 heads, head_dim 64, fp32.

Schedule: engine queues run in emission order, so the QKV projection of
sblock sb+1 and the out-projection of sblock sb-1 are emitted interleaved
into the (scalar-bound) attention groups of sblock sb to fill tensor-engine
gaps. Softmax 1/Z uses reciprocal_approx_fast + DMA partition-broadcast.
Exp is fused over 2-key-chunk score groups. The two heads of a pair run
their K=64 score matmuls concurrently on disjoint PE row halves.
"""

import sys
import numpy as np

if "/opt/trn_rl_repo" not in sys.path:
    sys.path.insert(0, "/opt/trn_rl_repo")

B = 4
S = 2048
D = 1024
HG = 2            # head groups (cores per batch)
NHL = 8           # heads per core
DH = 64
DG = NHL * DH     # 512 feature dims per core
SB = 512          # s-block
NSB = S // SB     # 4
NEG = -1.0e9
SCALE = 0.125     # 1/sqrt(64)

# normalization path: "fast" = reciprocal_approx_fast + gpsimd partition
# broadcast; "mm" = exact reciprocal + matmul broadcast (baseline style)
FAST_RECIP = True
FAST_BCAST = True

_CACHE = {}


def _build_nc():
    import concourse.bass as bass
    import concourse.bacc as bacc
    import concourse.tile as tile
    from concourse import mybir
    from contextlib import ExitStack

    f32 = mybir.dt.float32
    bf16 = mybir.dt.bfloat16
    AF = mybir.ActivationFunctionType
    ts = bass.ts

    nc = bacc.Bacc(None, target_bir_lowering=False)

    # tile-major DRAM layouts: each SBUF tile loads as ONE contiguous DMA
    # with 8 KB per-partition lines (fragmented layouts cost ~20 us startup)
    xt_d = nc.dram_tensor("xt", [NSB, 128, 8, SB], bf16, kind="ExternalInput")
    wqt_d = nc.dram_tensor("wqt", [128, 8, DG], bf16, kind="ExternalInput")
    wkt_d = nc.dram_tensor("wkt", [128, 8, DG], bf16, kind="ExternalInput")
    wvt_d = nc.dram_tensor("wvt", [128, 8, DG], bf16, kind="ExternalInput")
    wot_d = nc.dram_tensor("wot", [128, 4, D], bf16, kind="ExternalInput")
    bqc_d = nc.dram_tensor("bqc", [128, 4], f32, kind="ExternalInput")
    bkc_d = nc.dram_tensor("bkc", [128, 4], f32, kind="ExternalInput")
    bvb_d = nc.dram_tensor("bvb", [128, NHL, DH], f32, kind="ExternalInput")
    # outputs in bf16: halves the output DMA bytes; host sums/upcasts
    out_d = nc.dram_tensor("out", [S, D], bf16, kind="ExternalOutput")
    # per-pair out-projection partials for the last sblock (pairs 0-2);
    # the host adds them into out rows [1536:2048]
    o3p_d = nc.dram_tensor("o3p", [3, SB, D], bf16, kind="ExternalOutput")

    with tile.TileContext(nc) as tc, ExitStack() as ctx:
        consts = ctx.enter_context(tc.tile_pool(name="consts", bufs=1))
        cache = ctx.enter_context(tc.tile_pool(name="cache", bufs=1))
        xt_pool = ctx.enter_context(tc.tile_pool(name="xtp", bufs=2))
        qt_pool = ctx.enter_context(tc.tile_pool(name="qtp", bufs=2))
        work = ctx.enter_context(tc.tile_pool(name="work", bufs=1))
        ppool = ctx.enter_context(tc.tile_pool(name="pp", bufs=2, space="PSUM"))
        pscore = ctx.enter_context(tc.tile_pool(name="ps", bufs=2, space="PSUM"))
        pout2 = ctx.enter_context(tc.tile_pool(name="po", bufs=2, space="PSUM"))

        # ---- x for sblock 0 first so the first projection can start early ----
        xt_tiles = {}

        # spread input DMAs over several engines' DMA queues so the startup
        # load (~5 MB) doesn't serialize on one queue
        dmaq = [nc.sync, nc.scalar, nc.gpsimd]

        def load_xt(sb):
            xt_tiles[sb] = xt_pool.tile(
                [128, 8, SB], bf16, tag="xt", name=f"xt{sb}"
            )
            nc.sync.dma_start(xt_tiles[sb], xt_d[sb, :, :, :])

        # startup is DMA-bandwidth-bound: issue the critical tensors first
        # (xt0 + wk feed the first K-projection chains), split into ec-pieces
        # so the first chain starts after the first 2-chunk piece lands.
        xt_tiles[0] = xt_pool.tile([128, 8, SB], bf16, tag="xt", name="xt0")
        wk_t = consts.tile([128, 8, DG], bf16)
        wq_t = consts.tile([128, 8, DG], bf16)
        wv_t = consts.tile([128, 8, DG], bf16)
        for e0 in range(0, 8, 2):
            nc.sync.dma_start(xt_tiles[0][:, e0 : e0 + 2, :], xt_d[0, :, e0 : e0 + 2, :])
            nc.scalar.dma_start(wk_t[:, e0 : e0 + 2, :], wkt_d[:, e0 : e0 + 2, :])
        bkc_t = consts.tile([128, 4], f32)
        nc.gpsimd.dma_start(bkc_t, bkc_d[:, :])
        bqc_t = consts.tile([128, 4], f32)
        nc.gpsimd.dma_start(bqc_t, bqc_d[:, :])
        for e0 in range(0, 8, 2):
            nc.scalar.dma_start(wq_t[:, e0 : e0 + 2, :], wqt_d[:, e0 : e0 + 2, :])
        for e0 in range(0, 8, 2):
            nc.sync.dma_start(wv_t[:, e0 : e0 + 2, :], wvt_d[:, e0 : e0 + 2, :])
        bv_bc = consts.tile([128, NHL, DH], f32)
        nc.gpsimd.dma_start(bv_bc, bvb_d[:, :, :])
        # wo is first needed by the out-projection of sblock 0, deferred into
        # sblock 2's attention — load it once the startup burst has drained
        wo_t = consts.tile([128, 4, D], bf16)

        def load_wo():
            nc.gpsimd.dma_start(wo_t, wot_d[:, :, :])

        ones64 = consts.tile([65, 64], bf16)
        nc.any.memset(ones64, 1.0)

        # diag mask: m128[tt, c] = 0 if c >= tt else NEG
        m128 = consts.tile([128, 128], f32)
        nc.any.memset(m128, 0.0)
        nc.gpsimd.affine_select(
            out=m128,
            in_=m128,
            compare_op=mybir.AluOpType.is_ge,
            fill=NEG,
            base=0,
            pattern=[[1, 128]],
            channel_multiplier=-1,
        )
        # bf16 copies for adding the mask via a tensor-engine accumulation
        # (identity.T @ m128) instead of a vector add on the exp critical path
        m128b = consts.tile([128, 128], bf16)
        nc.vector.tensor_copy(m128b, m128)
        ident = consts.tile([128, 128], f32)
        nc.any.memset(ident, 1.0)
        nc.gpsimd.affine_select(
            out=ident, in_=ident, compare_op=mybir.AluOpType.is_ge,
            fill=0.0, base=0, pattern=[[1, 128]], channel_multiplier=-1,
        )
        nc.gpsimd.affine_select(
            out=ident, in_=ident, compare_op=mybir.AluOpType.is_ge,
            fill=0.0, base=0, pattern=[[-1, 128]], channel_multiplier=1,
        )
        identb = consts.tile([128, 128], bf16)
        nc.vector.tensor_copy(identb, ident)
        # dummy exp to pull the ACT table load into the startup phase
        exwarm = consts.tile([1, 8], bf16)
        nc.scalar.activation(exwarm, m128[0:1, 0:8], AF.Exp, scale=SCALE)

        # ---- persistent K/V caches ----
        kt_all = cache.tile([128, 4, S], bf16)       # [d within pair chunk, pair, t]
        v_aug = cache.tile([128, 16, NHL, DH + 1], bf16)  # [t in chunk, tchunk, head, d|1]
        nc.any.memset(v_aug[:, :, :, DH : DH + 1], 1.0)

        qt_tiles = {}

        def proj_closures(sb):
            """12 tensor-work closures: Q, K (4 dim-chunks each), V (4 t-chunks)."""
            s0 = sb * SB
            qt_tiles[sb] = qt_pool.tile([128, 4, SB], bf16, tag="qt", name=f"qt{sb}")
            xt_sb = xt_tiles[sb]
            items = []

            def q_group(dc):
                def emit():
                    pq = ppool.tile([128, SB], f32, tag="pp")
                    for ec in range(8):
                        nc.tensor.matmul(
                            pq, wq_t[:, ec, ts(dc, 128)], xt_sb[:, ec, :],
                            start=(ec == 0), stop=(ec == 7),
                        )
                    nc.vector.tensor_scalar_add(
                        qt_tiles[sb][:, dc, :], pq, bqc_t[:, dc : dc + 1]
                    )
                return emit

            def k_group(dc):
                def emit():
                    pk = ppool.tile([128, SB], f32, tag="pp")
                    for ec in range(8):
                        nc.tensor.matmul(
                            pk, wk_t[:, ec, ts(dc, 128)], xt_sb[:, ec, :],
                            start=(ec == 0), stop=(ec == 7),
                        )
                    nc.vector.tensor_scalar_add(
                        kt_all[:, dc, s0 : s0 + SB], pk, bkc_t[:, dc : dc + 1]
                    )
                return emit

            def v_group(tsub):
                def emit():
                    tcg = 4 * sb + tsub
                    pv = ppool.tile([128, NHL, DH], f32, tag="pp")
                    for ec in range(8):
                        nc.tensor.matmul(
                            pv, xt_sb[:, ec, ts(tsub, 128)], wv_t[:, ec, :],
                            start=(ec == 0), stop=(ec == 7),
                        )
                    nc.vector.tensor_add(v_aug[:, tcg, :, 0:DH], pv, bv_bc)
                return emit

            # K first (needed by scores of the first groups), then Q, then V
            for dc in range(4):
                items.append(k_group(dc))
            for dc in range(4):
                items.append(q_group(dc))
            for tsub in range(4):
                items.append(v_group(tsub))
            return items

        ao_tiles_by_sb = {}

        def outproj_closures(sb):
            """8 tensor-work closures: 4 s-chunks x 2 output halves."""
            s0 = sb * SB
            ao_tiles = ao_tiles_by_sb[sb]
            items = []

            def o_group(sc, oh):
                def emit():
                    po = ppool.tile([128, 512], f32, tag="pp")
                    for p in range(4):
                        nc.tensor.matmul(
                            po,
                            ao_tiles[p][:, ts(sc, 128)],
                            wo_t[:, p, ts(oh, 512)],
                            start=(p == 0), stop=(p == 3),
                        )
                    po_sb = work.tile([128, 512], bf16, tag="posb", bufs=4)
                    nc.vector.tensor_copy(po_sb, po)
                    nc.sync.dma_start(
                        out_d[s0 + 128 * sc : s0 + 128 * (sc + 1), ts(oh, 512)], po_sb
                    )
                return emit

            for sc in range(4):
                for oh in range(2):
                    items.append(o_group(sc, oh))
            return items

        # sblock 0 projections run up front
        for it in proj_closures(0):
            it()

        for sb in range(NSB):
            s0 = sb * SB
            nkc = 4 * sb + 4
            ngrp = nkc // 2
            qt_sb = qt_tiles[sb]

            # deferred tensor work to interleave into this sblock's attention
            # rebalance: early sblocks are tensor-rich (projections), late
            # sblocks scalar-rich (attention) — push out-projections two
            # sblocks later so sb3's 32 groups get tensor filler.
            deferred = []
            if sb == 0:
                load_wo()
            if sb + 1 < NSB:
                load_xt(sb + 1)
                deferred += proj_closures(sb + 1)
            if sb == 2:
                deferred += outproj_closures(0)
            elif sb == 3:
                deferred += outproj_closures(1)
                deferred += outproj_closures(2)
            total_groups = ngrp * 4
            emitted = [0]
            gidx = [0]
            # reserve up to 2 items per pair boundary: the normalization
            # chain there leaves the PE idle long enough to re-throttle HAM
            n_boundary = min(len(deferred), 8)
            n_paced = len(deferred) - n_boundary

            def pace():
                gidx[0] += 1
                want = n_paced * gidx[0] // total_groups
                while emitted[0] < want:
                    deferred[emitted[0]]()
                    emitted[0] += 1

            def pace_boundary():
                for _ in range(2):
                    if emitted[0] < len(deferred):
                        deferred[emitted[0]]()
                        emitted[0] += 1

            def flush_deferred():
                while emitted[0] < len(deferred):
                    deferred[emitted[0]]()
                    emitted[0] += 1

            # ---- attention, per head-pair; key chunks in groups of 2 ----
            for p in range(4):
                out2 = [
                    pout2.tile([DH + 1, SB], f32, tag="po", name=f"out2_{hh}")
                    for hh in range(2)
                ]
                prev = None  # (ex tiles per hh, [c0 per chunk], [kc per chunk])
                for g in range(ngrp):
                    kcs = [2 * g, 2 * g + 1]
                    c0s = [max(0, 128 * (kc - 4 * sb)) for kc in kcs]
                    diag = kcs[1] >= 4 * sb
                    cur_st = []
                    cur_ex = []
                    # scores: 2 chunks x 2 heads; the two heads' matmuls use
                    # disjoint PE row halves (K=64) and run concurrently.
                    for hh in range(2):
                        r0 = 64 * hh
                        st = pscore.tile([128, 2, SB], f32, tag="ps")
                        cur_st.append(st)
                        for i in range(2):
                            nc.tensor.matmul(
                                st[:, i, c0s[i] : SB],
                                kt_all[r0 : r0 + 64, p, ts(kcs[i], 128)],
                                qt_sb[r0 : r0 + 64, p, c0s[i] : SB],
                                start=True, stop=True,
                                tile_position=(r0, 0),
                            )
                    # interleave: attn@V for the previous group
                    if prev is not None:
                        pex, pc0s, pkcs = prev
                        for i in range(2):
                            for hh in range(2):
                                nc.tensor.matmul(
                                    out2[hh][:, pc0s[i] : SB],
                                    v_aug[:, pkcs[i], 2 * p + hh, :],
                                    pex[hh][:, i, pc0s[i] : SB],
                                    start=(pkcs[i] == 0), stop=False,
                                )
                    for hh in range(2):
                        st = cur_st[hh]
                        ex = work.tile([128, 2, SB], bf16, tag="expt", bufs=4)
                        if diag:
                            for i in range(2):
                                nc.vector.tensor_add(
                                    st[:, i, c0s[i] : c0s[i] + 128],
                                    st[:, i, c0s[i] : c0s[i] + 128],
                                    m128,
                                )
                                nc.scalar.activation(
                                    ex[:, i, c0s[i] : SB],
                                    st[:, i, c0s[i] : SB],
                                    AF.Exp, scale=SCALE,
                                )
                        else:
                            nc.scalar.activation(
                                ex[:, :, :], st[:, :, :], AF.Exp, scale=SCALE
                            )
                        cur_ex.append(ex)
                    prev = (cur_ex, c0s, kcs)
                    pace()
                # final attn@V for the last group
                pex, pc0s, pkcs = prev
                for i in range(2):
                    for hh in range(2):
                        nc.tensor.matmul(
                            out2[hh][:, pc0s[i] : SB],
                            v_aug[:, pkcs[i], 2 * p + hh, :],
                            pex[hh][:, i, pc0s[i] : SB],
                            start=(pkcs[i] == 0), stop=(i == 1),
                        )
                if p == 3:
                    # everything left must land before the serial tail
                    flush_deferred()

                # ---- normalization: 1/Z (fast approx), DMA partition-bcast, mul
                # phases interleaved across the two heads so vector and
                # gpsimd pipeline instead of serializing the chain twice
                ao_p = work.tile([128, SB], bf16, tag=f"ao{p}", bufs=3)
                zcps, rzs, bcs = [], [], []
                for hh in range(2):
                    # stage Z into SBUF at partition 0: the custom-DVE ucode
                    # mishandles PSUM / nonzero-base inputs
                    zcp = work.tile([1, SB], f32, tag=f"zcp{hh}", bufs=2)
                    nc.vector.tensor_copy(zcp[0:1, :], out2[hh][DH : DH + 1, :])
                    zcps.append(zcp)
                for hh in range(2):
                    rz = work.tile([1, SB], f32, tag=f"rz{hh}", bufs=2)
                    nc.vector.reciprocal_approx_fast(
                        out=rz[0:1, :], in_=zcps[hh][0:1, :]
                    )
                    rzs.append(rz)
                for hh in range(2):
                    bc = work.tile([64, SB], f32, tag=f"bc{hh}", bufs=2)
                    nc.gpsimd.partition_broadcast(bc, rzs[hh][0:1, :])
                    bcs.append(bc)
                nc.vector.tensor_mul(ao_p[0:64, :], out2[0][0:DH, :], bcs[0])
                aotmp = work.tile([64, SB], bf16, tag="aotmp", bufs=2)
                nc.vector.tensor_mul(aotmp, out2[1][0:DH, :], bcs[1])
                nc.gpsimd.dma_start(ao_p[64:128, :], aotmp)
                if p == 0:
                    ao_tiles_by_sb[sb] = []
                ao_tiles_by_sb[sb].append(ao_p)
                if p < 3:
                    pace_boundary()
                if sb == NSB - 1:
                    # last sblock: out-projection per pair, right after the
                    # pair's normalization — kills the serial tail. Pairs
                    # 0-2 go to o3p_d; the host sums them into out.
                    for sc in range(4):
                        for oh in range(2):
                            po = ppool.tile([128, 512], f32, tag="pp")
                            nc.tensor.matmul(
                                po,
                                ao_p[:, ts(sc, 128)],
                                wo_t[:, p, ts(oh, 512)],
                                start=True, stop=True,
                            )
                            po_sb = work.tile([128, 512], bf16, tag="posb", bufs=4)
                            if oh == 1:
                                # split evacuation across scalar and vector so
                                # the psum-buffer rotation doesn't drip at
                                # single-engine copy pace
                                nc.scalar.copy(po_sb, po)
                            else:
                                nc.vector.tensor_copy(po_sb, po)
                            if p == 3:
                                nc.sync.dma_start(
                                    out_d[s0 + 128 * sc : s0 + 128 * (sc + 1),
                                          ts(oh, 512)],
                                    po_sb,
                                )
                            else:
                                nc.gpsimd.dma_start(
                                    o3p_d[p, 128 * sc : 128 * (sc + 1), ts(oh, 512)],
                                    po_sb,
                                )

            # any stragglers not yet emitted by pacing
            while emitted[0] < len(deferred):
                deferred[emitted[0]]()
                emitted[0] += 1



    nc.compile()
    return nc


def _prepare_core_inputs(x, Wq, bq, Wk, bk, Wv, bv):
    """Build per-core input maps. Core c: b = c // 2, g = c % 2."""
    import ml_dtypes

    BF = ml_dtypes.bfloat16

    def tilemajor(wT):
        # [D, F] -> [128, D//128, F]: partition-major so each SBUF tile
        # loads as one contiguous DMA
        F = wT.shape[1]
        return np.ascontiguousarray(
            wT.reshape(-1, 128, F).transpose(1, 0, 2)
        ).astype(BF)

    maps = []
    # x^T [1024, 2048] -> [NSB, 128, 8, SB]
    xt = []
    for b in range(B):
        xT = x[b].T  # [D, S]
        xt.append(np.ascontiguousarray(
            xT.reshape(8, 128, NSB, SB).transpose(2, 1, 0, 3)
        ).astype(BF))
    wq_s, wk_s, wv_s, bq_s, bk_s, bv_s = [], [], [], [], [], []
    for g in range(HG):
        sl = slice(g * DG, (g + 1) * DG)
        wq_s.append(tilemajor(Wq[sl, :].T))
        wk_s.append(tilemajor(Wk[sl, :].T))
        wv_s.append(tilemajor(Wv[sl, :].T))
        # Q/K biases in per-partition column layout [128, 4]: bias for dim
        # 128*dc + d sits at [d, dc].
        bq_s.append(np.ascontiguousarray(bq[sl].reshape(4, 128).T).astype(np.float32))
        bk_s.append(np.ascontiguousarray(bk[sl].reshape(4, 128).T).astype(np.float32))
        # V bias pre-broadcast across partitions: [128, NHL, DH]
        bv_s.append(
            np.ascontiguousarray(
                np.broadcast_to(bv[sl].reshape(1, NHL, DH), (128, NHL, DH))
            ).astype(np.float32)
        )
    for c in range(B * HG):
        b, g = c // HG, c % HG
        maps.append({
            "xt": xt[b],
            "wqt": wq_s[g], "wkt": wk_s[g], "wvt": wv_s[g],
            "wot": None,  # filled by caller (needs Wo)
            "bqc": bq_s[g], "bkc": bk_s[g], "bvb": bv_s[g],
        })
    return maps


def kernel(x, Wq, bq, Wk, bk, Wv, bv, Wo, bo):
    from concourse.bass_utils import run_bass_kernel_spmd

    x = np.asarray(x, dtype=np.float32)
    Wq, bq = np.asarray(Wq, np.float32), np.asarray(bq, np.float32)
    Wk, bk = np.asarray(Wk, np.float32), np.asarray(bk, np.float32)
    Wv, bv = np.asarray(Wv, np.float32), np.asarray(bv, np.float32)
    Wo, bo = np.asarray(Wo, np.float32), np.asarray(bo, np.float32)

    if "nc" not in _CACHE:
        _CACHE["nc"] = _build_nc()
    nc = _CACHE["nc"]

    import ml_dtypes

    maps = _prepare_core_inputs(x, Wq, bq, Wk, bk, Wv, bv)
    wot = [
        np.ascontiguousarray(
            Wo.T[g * DG : (g + 1) * DG, :]
            .reshape(4, 128, D)
            .transpose(1, 0, 2)
        ).astype(ml_dtypes.bfloat16)
        for g in range(HG)
    ]
    for c in range(B * HG):
        maps[c]["wot"] = wot[c % HG]

    res = run_bass_kernel_spmd(nc, maps, list(range(B * HG)))
    _CACHE["last_results"] = res

    out = np.empty((B, S, D), dtype=np.float32)
    for b in range(B):
        r0, r1 = res.results[2 * b], res.results[2 * b + 1]
        out[b] = (
            r0["out"].astype(np.float32) + r1["out"].astype(np.float32) + bo
        )
        out[b][3 * SB :] += (
            r0["o3p"].astype(np.float32).sum(axis=0)
            + r1["o3p"].astype(np.float32).sum(axis=0)
        )
    return out
